# revision 1
# baseline (speedup 1.0000x reference)
"""Expert-parallel Trainium2 Bass kernel for DeepEquiCategorySpecificMLP.

Routing strategy (host side): tokens are sorted by cat_id; core c receives
all tokens of category c (padded to a fixed PAD) plus that category's
weight stack. All compute (input LN, 5 matmuls, gated MLP, 3 more LNs,
residual) runs on-device in a feature-major layout ([feature, token]), so
every matmul consumes activations directly as the moving operand with the
weight stack as the stationary operand (out = W.T @ actT) and no on-device
transposes are needed. LayerNorm is over the feature axis = partition axis:
sums are computed on the TensorEngine (ones-vector matmuls accumulating in
PSUM), rstd is computed as exp(-0.5*ln(var)) on the ScalarEngine, per-token
scale/shift rows are broadcast across partitions on GPSIMD, and applied on
the VectorEngine.
"""


import numpy as np
from contextlib import ExitStack

N_CORES = 8
D = 256
H = 1024
EPS = 1e-5
PAD_MIN = 288  # >= max per-category count (283 at seed 0); >=256 keeps f32r matmuls full-rate

# Experiment knobs
MM_DTYPE = "bf16"  # "f32r" | "bf16"
BCAST = "pe"   # "gpsimd" | "pe"

_cache = {}


def _build(PAD, center_only_gln, zero_b2=True):
    import concourse.bass as bass
    import concourse.tile as tile
    from concourse import bacc, mybir

    f32 = mybir.dt.float32
    f32r = mybir.dt.float32r
    mmdt = mybir.dt.bfloat16 if MM_DTYPE == "bf16" else f32r
    # dtype for the output pathway (y, residual, final LN) — always f32r
    # so the final LayerNorm sees full-precision inputs.
    odt = f32r
    AF = mybir.ActivationFunctionType
    ALU = mybir.AluOpType
    KD, KH = D // 128, H // 128
    NBIAS = 4 * KH + KD  # bias ball columns

    nc = bacc.Bacc("TRN2", target_bir_lowering=False, debug=False,
                   num_devices=N_CORES)

    xT_d = nc.dram_tensor("xT", [D, PAD], odt, kind="ExternalInput")
    w0_d = nc.dram_tensor("W0", [D, H], mmdt, kind="ExternalInput")
    wm_d = nc.dram_tensor("Wm", [H, H], mmdt, kind="ExternalInput")
    wg_d = nc.dram_tensor("Wg", [H, H], mmdt, kind="ExternalInput")
    wog_d = nc.dram_tensor("Wog", [H, H], mmdt, kind="ExternalInput")
    w2_d = nc.dram_tensor("W2", [H, D], odt, kind="ExternalInput")
    bias_d = nc.dram_tensor("bias", [128 * NBIAS], f32, kind="ExternalInput")
    out_d = nc.dram_tensor("outT", [D, PAD], f32, kind="ExternalOutput")

    with ExitStack() as ctx:
        tc = ctx.enter_context(tile.TileContext(nc))
        wp = ctx.enter_context(tc.tile_pool(name="w", bufs=1))
        ap_ = ctx.enter_context(tc.tile_pool(name="a", bufs=1))
        sqp = ctx.enter_context(tc.tile_pool(name="sq", bufs=3))
        stp = ctx.enter_context(tc.tile_pool(name="st", bufs=2))
        pmm = ctx.enter_context(
            tc.tile_pool(name="pmm", bufs=4, space=bass.MemorySpace.PSUM))
        pst = ctx.enter_context(
            tc.tile_pool(name="pst", bufs=2, space=bass.MemorySpace.PSUM))

        # ---- input DMA: few large descriptors, issued from two HWDGE
        # engines (sync + scalar) so descriptor generation is not serial.
        def load_merged(eng, dram, K, mfree, name):
            """[K*128, mfree] dram -> one [128, K*mfree] tile; view k-tiles."""
            t = wp.tile([128, K * mfree], mmdt, tag=name, name=name)
            eng.dma_start(
                t[:].rearrange("p (k m) -> p k m", k=K),
                dram.ap().rearrange("(k p) m -> p k m", p=128))
            return [t[:, k * mfree:(k + 1) * mfree] for k in range(K)]

        def load_pairs(eng, dram, K, mfree, tagp, dt_):
            tiles = []
            for j in range(K // 2):
                t = wp.tile([128, 2 * mfree], dt_, tag=f"{tagp}{j}",
                            name=f"{tagp}{j}")
                eng.dma_start(
                    t[:].rearrange("p (k m) -> p k m", k=2),
                    dram.ap()[j * 256:(j + 1) * 256, :].rearrange(
                        "(k p) m -> p k m", p=128))
                tiles.append(t[:, 0:mfree])
                tiles.append(t[:, mfree:2 * mfree])
            return tiles

        def load_2d(eng, dram, K, mfree, tagp, dt_):
            tiles = []
            for k in range(K):
                t = wp.tile([128, mfree], dt_, tag=f"{tagp}{k}",
                            name=f"{tagp}{k}")
                eng.dma_start(t[:], dram.ap()[k * 128:(k + 1) * 128, :])
                tiles.append(t)
            return tiles

        xT = load_2d(nc.sync, xT_d, KD, PAD, "xT", odt)
        bias_t = wp.tile([128, NBIAS], f32, tag="bias", name="bias")
        nc.sync.dma_start(bias_t[:],
                          bias_d.ap().rearrange("(j p) -> p j", p=128))
        w0 = load_2d(nc.sync, w0_d, KD, H, "w0", mmdt)
        b0t = bias_t[:, 0:KH]
        bmt = bias_t[:, KH:2 * KH]
        bgt = bias_t[:, 2 * KH:3 * KH]
        bogt = bias_t[:, 3 * KH:4 * KH]
        b2t = bias_t[:, 4 * KH:4 * KH + KD]

        wm = load_pairs(nc.sync, wm_d, KH, H, "wm", mmdt)
        wg = load_pairs(nc.sync, wg_d, KH, H, "wg", mmdt)
        wog = load_pairs(nc.sync, wog_d, KH, H, "wog", mmdt)
        w2 = load_2d(nc.sync, w2_d, KH, D, "w2", odt)

        onesf = wp.tile([128, 1], f32, tag="onesf", name="onesf")
        nc.vector.memset(onesf[:], 1.0)
        onesc = wp.tile([128, 1], mmdt, tag="ones", name="ones")
        nc.vector.tensor_copy(onesc[:], onesf[:])
        if mmdt != odt:
            oneso = wp.tile([128, 1], odt, tag="oneso", name="oneso")
            nc.vector.tensor_copy(oneso[:], onesf[:])
        else:
            oneso = onesc
        if BCAST == "pe":
            onesr = wp.tile([1, 128], f32r, tag="onesr", name="onesr")
            nc.vector.tensor_copy(onesr[:], onesf[:1, :].broadcast_to([1, 128]))
        # per-F eps bias for the rsqrt input
        eps_t = {}
        for F in (D, H):
            t = wp.tile([1, 1], f32, tag=f"eps{F}", name=f"eps{F}")
            nc.vector.memset(t[:], float(F) * float(F) * EPS)
            eps_t[F] = t

        def stats_sum(x_tiles, ones):
            s = pst.tile([1, PAD], f32, tag="st", name="stat")
            K = len(x_tiles)
            for k in range(K):
                nc.tensor.matmul(s[:], ones[:], x_tiles[k][:],
                                 start=(k == 0), stop=(k == K - 1))
            return s

        def stats_sumsq(x_tiles, ones, dt_):
            s = pst.tile([1, PAD], f32, tag="st", name="stat")
            K = len(x_tiles)
            for k in range(K):
                sqt = sqp.tile([128, PAD], dt_, tag="sqt", name="sqt")
                nc.vector.tensor_mul(sqt[:], x_tiles[k][:], x_tiles[k][:])
                nc.tensor.matmul(s[:], ones[:], sqt[:],
                                 start=(k == 0), stop=(k == K - 1))
            return s

        def bcast(src_row, tag, btag="bcA"):
            if BCAST == "gpsimd":
                b = ap_.tile([128, PAD], f32, tag=btag, name=tag, bufs=2)
                nc.gpsimd.partition_broadcast(b[:], src_row[:])
            else:
                b = pmm.tile([128, PAD], f32, tag="bc", name=tag, bufs=2)
                nc.tensor.matmul(b[:], onesr[:], src_row[:],
                                 start=True, stop=True)
            return b

        def ln_full(x_tiles, F, pref, ones, dt_):
            """LN stats over the partition (feature) axis.

            Returns (A_b, B_b) with normalized = x*A_b + B_b where
            A = rstd = F * (F*s2 - s1^2 + F^2*eps)^-1/2 computed via
            exp(ln(F) - 0.5*ln(u)), B = -(s1/F)*A.
            """
            s1 = stats_sum(x_tiles, ones)
            s2 = stats_sumsq(x_tiles, ones, dt_)
            s1s = stp.tile([1, PAD], f32, tag="st_s1", name=f"{pref}s1")
            nc.vector.tensor_copy(s1s[:], s1[:])
            t1 = stp.tile([1, PAD], f32, tag="st_t1", name=f"{pref}t1")
            nc.vector.tensor_mul(t1[:], s1s[:], s1s[:])
            u = stp.tile([1, PAD], f32, tag="st_u", name=f"{pref}u")
            nc.vector.scalar_tensor_tensor(u[:], s2[:], float(F), t1[:],
                                           op0=ALU.mult, op1=ALU.subtract)
            # r = (u + F^2 eps)^-1/2 ; rstd = F*r (F folded into the apply)
            rr = stp.tile([1, PAD], f32r, tag="st_A", name=f"{pref}A")
            nc.scalar.activation(rr[:], u[:], AF.Abs_reciprocal_sqrt,
                                 bias=eps_t[F][:])
            Bs = stp.tile([1, PAD], f32r, tag="st_Bs", name=f"{pref}Bs")
            nc.vector.scalar_tensor_tensor(Bs[:], s1s[:], -1.0, rr[:],
                                           op0=ALU.mult, op1=ALU.mult)
            return bcast(rr, f"{pref}Ab", "bcA"), bcast(Bs, f"{pref}Bb", "bcB")

        def apply_full(x_k, out_k, F, Ab, Bb):
            nc.vector.scalar_tensor_tensor(out_k[:], x_k[:], float(F), Ab[:],
                                           op0=ALU.mult, op1=ALU.mult)
            nc.vector.tensor_add(out_k[:], out_k[:], Bb[:])

        def mm_layer(wtiles, atiles, K, MT, mgroup, evac):
            outs = []
            for g0 in range(0, MT, mgroup):
                ms = list(range(g0, min(g0 + mgroup, MT)))
                pss = [pmm.tile([128, PAD], f32, tag="mmps", name="mmps")
                       for _ in ms]
                for k in range(K):
                    for i, m in enumerate(ms):
                        nc.tensor.matmul(
                            pss[i][:],
                            wtiles[k][:, m * 128:(m + 1) * 128],
                            atiles[k][:],
                            start=(k == 0), stop=(k == K - 1))
                for i, m in enumerate(ms):
                    outs.append(evac(m, pss[i]))
            return outs

        def evac_act(func, bias_tile, tagp, dt_):
            def f(m, ps):
                t = ap_.tile([128, PAD], dt_, tag=f"{tagp}{m}",
                             name=f"{tagp}{m}")
                nc.scalar.activation(t[:], ps[:], func,
                                     bias=bias_tile[:, m:m + 1])
                return t
            return f

        # ---- input LN over D ----
        Ab, Bb = ln_full(xT, D, "iln", oneso, odt)
        xn = []
        for k in range(KD):
            t = ap_.tile([128, PAD], mmdt, tag=f"xn{k}", name=f"xn{k}")
            apply_full(xT[k], t, D, Ab, Bb)
            xn.append(t)

        # ---- h = relu(xn @ W0 + b0) ----
        h = mm_layer(w0, xn, KD, KH, 4, evac_act(AF.Relu, b0t, "h", mmdt))

        # ---- main/gate, gated = main * sigmoid(gate) ----
        mainT = mm_layer(wm, h, KH, KH, 4,
                         evac_act(AF.Identity, bmt, "mn", mmdt))
        sigT = mm_layer(wg, h, KH, KH, 4,
                        evac_act(AF.Sigmoid, bgt, "sg", mmdt))
        for k in range(KH):
            nc.vector.tensor_mul(mainT[k][:], mainT[k][:], sigT[k][:])

        # ---- g = LN(gated): when bog == 0 the per-token scale washes out in
        # the next LN, so only centering is required.
        if center_only_gln:
            s1 = stats_sum(mainT, onesc)
            Bs = stp.tile([1, PAD], f32r, tag="st_Bs", name="glBs")
            nc.vector.tensor_scalar_mul(Bs[:], s1[:], -1.0 / float(H))
            Bb1 = bcast(Bs, "glBb", "bcB")
            for k in range(KH):
                nc.vector.tensor_add(mainT[k][:], mainT[k][:], Bb1[:])
        else:
            Ab1, Bb1 = ln_full(mainT, H, "gln", onesc, mmdt)
            for k in range(KH):
                apply_full(mainT[k], mainT[k], H, Ab1, Bb1)

        # ---- h2 = LN(g @ Wog + bog): center immediately so mm2 can start;
        # the per-token scale rstd2 = H*r2 is applied to y afterwards
        # (exact: (c*h2c) @ W2 = c * (h2c @ W2) per token).
        h2 = mm_layer(wog, mainT, KH, KH, 4,
                      evac_act(AF.Identity, bogt, "h2", odt))
        s1h = stats_sum(h2, oneso)
        s2h = stats_sumsq(h2, oneso, odt)
        s1hs = stp.tile([1, PAD], f32, tag="st_s1", name="hlns1")
        nc.vector.tensor_copy(s1hs[:], s1h[:])
        Bch = stp.tile([1, PAD], f32r, tag="st_Bs", name="hlnBc")
        nc.vector.tensor_scalar_mul(Bch[:], s1hs[:], -1.0 / float(H))
        Bb2 = bcast(Bch, "hlnBb", "bcB")
        for k in range(KH):
            nc.vector.tensor_add(h2[k][:], h2[k][:], Bb2[:])
        # r2 chain (overlaps mm2 on the PE)
        t1h = stp.tile([1, PAD], f32, tag="st_t1", name="hlnt1")
        nc.vector.tensor_mul(t1h[:], s1hs[:], s1hs[:])
        uh = stp.tile([1, PAD], f32, tag="st_u", name="hlnu")
        nc.vector.scalar_tensor_tensor(uh[:], s2h[:], float(H), t1h[:],
                                       op0=ALU.mult, op1=ALU.subtract)
        r2 = stp.tile([1, PAD], f32r, tag="st_A", name="hlnr2")
        nc.scalar.activation(r2[:], uh[:], AF.Abs_reciprocal_sqrt,
                             bias=eps_t[H][:])
        # r2b must live in SBUF (evac_y also reads the matmul PSUM) —
        # broadcast on GPSIMD which writes SBUF.
        r2b = ap_.tile([128, PAD], f32r, tag="r2b", name="r2b")
        nc.gpsimd.partition_broadcast(r2b[:], r2[:])

        # ---- y = (h2c @ W2) * (H*r2) + b2 ; out = LN(y + 0.1 x) ----
        have_b2 = not zero_b2

        def evac_y(m, ps):
            t = ap_.tile([128, PAD], f32, tag=f"y{m}", name=f"y{m}")
            # (mm * H) * r2b  — per-token rescale fused with PSUM evacuation
            nc.vector.scalar_tensor_tensor(t[:], ps[:], float(H), r2b[:],
                                           op0=ALU.mult, op1=ALU.mult)
            return t

        y = mm_layer(w2, h2, KH, KD, 2, evac_y)
        opre = []
        for k in range(KD):
            yk = y[k]
            if have_b2:
                nc.vector.tensor_scalar(yk[:], yk[:], b2t[:, k:k + 1], None,
                                        op0=ALU.add)
            t = ap_.tile([128, PAD], odt, tag=f"op{k}", name=f"op{k}")
            nc.vector.scalar_tensor_tensor(t[:], xT[k][:], 0.1, yk[:],
                                           op0=ALU.mult, op1=ALU.add)
            opre.append(t)
        Ab3, Bb3 = ln_full(opre, D, "oln", oneso, odt)
        for k in range(KD):
            ot = ap_.tile([128, PAD], f32, tag=f"ot{k}", name=f"ot{k}")
            apply_full(opre[k], ot, D, Ab3, Bb3)
            nc.sync.dma_start(out_d.ap()[k * 128:(k + 1) * 128, :], ot[:])

    nc.compile()
    return nc


def _get_nc(PAD, center_only_gln, zero_b2=True):
    key = (PAD, center_only_gln, zero_b2, MM_DTYPE, BCAST)
    if key not in _cache:
        _cache[key] = _build(PAD, center_only_gln, zero_b2)
    return _cache[key]


def _np_mmdt():
    if MM_DTYPE == "bf16":
        import ml_dtypes
        return ml_dtypes.bfloat16
    return np.float32


def _prep(x, cat_ids, W0, b0, Wm, bm, Wg, bg, Wog, bog, W2, b2):
    x = np.ascontiguousarray(np.asarray(x, dtype=np.float32))
    cid = np.asarray(cat_ids).astype(np.int64).ravel()
    counts = np.bincount(cid, minlength=N_CORES)
    PAD = int(max(PAD_MIN, ((counts.max() + 31) // 32) * 32))
    order = np.argsort(cid, kind="stable")
    starts = np.zeros(N_CORES + 1, np.int64)
    starts[1:] = np.cumsum(counts)
    np_dt = _np_mmdt()

    def cvt(a):
        return np.ascontiguousarray(
            np.asarray(a, dtype=np.float32).astype(np_dt))

    in_maps = []
    for c in range(N_CORES):
        ids = order[starts[c]:starts[c + 1]]
        xc = np.zeros((PAD, D), np.float32)
        xc[:len(ids)] = x[ids]
        bias_ball = np.concatenate([
            np.asarray(b0[c], np.float32).ravel(),
            np.asarray(bm[c], np.float32).ravel(),
            np.asarray(bg[c], np.float32).ravel(),
            np.asarray(bog[c], np.float32).ravel(),
            np.asarray(b2[c], np.float32).ravel(),
        ])
        in_maps.append({
            "xT": np.ascontiguousarray(xc.T),
            "W0": cvt(W0[c]), "Wm": cvt(Wm[c]), "Wg": cvt(Wg[c]),
            "Wog": cvt(Wog[c]),
            "W2": np.ascontiguousarray(np.asarray(W2[c], np.float32)),
            "bias": np.ascontiguousarray(bias_ball),
        })
    center_only = not np.any(np.asarray(bog))
    zero_b2 = not np.any(np.asarray(b2))
    return in_maps, order, starts, PAD, center_only, zero_b2, x.shape[0]


def kernel(x, cat_ids, W0, b0, Wm, bm, Wg, bg, Wog, bog, W2, b2, **run_kwargs):
    from concourse.bass_utils import run_bass_kernel_spmd

    in_maps, order, starts, PAD, center_only, zero_b2, N = _prep(
        x, cat_ids, W0, b0, Wm, bm, Wg, bg, Wog, bog, W2, b2)
    nc = _get_nc(PAD, center_only, zero_b2)
    res = run_bass_kernel_spmd(nc, in_maps, core_ids=list(range(N_CORES)),
                               **run_kwargs)
    out = np.zeros((N, D), np.float32)
    for c in range(N_CORES):
        ids = order[starts[c]:starts[c + 1]]
        out[ids] = res.results[c]["outT"].T[:len(ids)]
    if run_kwargs:
        kernel.last_results = res
    return out

